# revision 12
# baseline (speedup 1.0000x reference)
"""Group-equivariant depthwise conv (C4) on 8 Trainium2 NeuronCores.

out[b, r*C+c] = crosscorr(x[b, c], rot90(weight[c, 0], r)), r in 0..3
x: [16, 192, 128, 128] f32, weight: [192, 1, 3, 3] f32 -> out: [16, 768, 128, 128].

Sharding: data-parallel over batch (2 images per core); per core the (b, c)
rows flatten to 384 partition-rows = 3 chunks of 128.

Algorithm (C4 symmetry decompositions, all bf16 on chip):
  With pair-sum planes s_k = x_a + x_b and pair-difference planes
  d_k = x_a - x_b over the 4 centrosymmetric tap pairs:
    p  = (out0+out2)/2 = sum_k alpha_k  s_k + alpha_c x_c     (5 matmuls)
    m  = (out0-out2)/2 = sum_k beta_k   d_k                   (4 matmuls)
    p' = (out1+out3)/2 = sum_k alpha'_k s_k + alpha_c x_c     (5 matmuls)
    m' = (out1-out3)/2 = sum_k beta'_k  d_k                   (4 matmuls)
  -> 18 diagonal-stationary matmuls for all 4 rotations ("PM" tiles).
  On "F" tiles the symmetric half is folded further via the C4 Fourier
  basis (pi1 = s1+s3, pi2 = s2+s4, pi3 = s1-s3, pi4 = s2-s4):
    qt = (p+p')/2 = t1 pi1 + t2 pi2 + alpha_c x_c             (3 matmuls)
    qs = (p-p')/2 = g1 pi3 + g2 pi4                           (2 matmuls)
    p = qt+qs, p' = qt-qs
  -> 13 matmuls for all 4 rotations, trading TensorE feed (the bottleneck)
  for extra DVE bf16 adds. The Scalar engine drains PSUM chain-pairs to
  SBUF bf16; DVE does the final +/- combines.

Input x is zero-padded (H+2, W+2) and converted to bf16 on the host, so
there are no on-chip memsets and loads/stores are large contiguous DMAs.
Output is stored as bf16 and upcast to f32 on the host.
"""

import numpy as np
from contextlib import ExitStack

from concourse import bacc, mybir, tile
from concourse.bass_utils import run_bass_kernel_spmd

B, C, H, W = 16, 192, 128, 128
NCORES = 8
BS = B // NCORES            # batches per core
ROWS = BS * C               # 384 (b,c) rows per core
NCHUNK = ROWS // 128        # 3
HT = 16                     # output rows per h-tile
NHT = H // HT               # 8
SUB = 4                     # output rows per PSUM chain subtile
NSUB = HT // SUB            # 4
HP, WP = H + 2, W + 2       # host-padded image dims

F32 = mybir.dt.float32
BF16 = mybir.dt.bfloat16
NPBF16 = mybir.dt.np(BF16)

# centrosymmetric tap pairs (first, second) as (i, j) offsets in the 3x3 kernel
PAIRS = [((0, 0), (2, 2)), ((0, 1), (2, 1)), ((0, 2), (2, 0)), ((1, 2), (1, 0))]

# w23 column layout:
#   0..4   alpha_1..4, alpha_c          (p chain)
#   5..8   beta_1..4                    (m chain)
#   9..13  alpha'_1..4, alpha_c         (p' chain)
#   14..17 beta'_1..4                   (m' chain)
#   18..20 t1, t2, alpha_c              (qt chain, F tiles)
#   21..22 g1, g2                       (qs chain, F tiles)
NW = 23

# partition segments of each chunk: (p0, n, b_local, c0)
CHUNK_SEGS = []
for _ch in range(NCHUNK):
    segs = []
    g = _ch * 128
    while g < (_ch + 1) * 128:
        b_loc, c0 = g // C, g % C
        n = min((_ch + 1) * 128 - g, C - c0)
        segs.append((g - _ch * 128, n, b_loc, c0))
        g += n
    CHUNK_SEGS.append(segs)


def _is_f_tile(ch, ht):
    return False


def _build():
    nc = bacc.Bacc("TRN2", target_bir_lowering=False, debug=False, num_devices=NCORES)
    x_d = nc.dram_tensor("xp", [ROWS, HP, WP], BF16, kind="ExternalInput").ap()
    w_d = nc.dram_tensor("w23", [ROWS, NW], F32, kind="ExternalInput").ap()
    o_d = nc.dram_tensor("out", [BS * 4 * C, H, W], BF16, kind="ExternalOutput").ap()

    ADD = mybir.AluOpType.add
    SUBT = mybir.AluOpType.subtract

    with tile.TileContext(nc) as tc, ExitStack() as ctx:
        xpool = ctx.enter_context(tc.tile_pool(name="xraw", bufs=2))
        spool = ctx.enter_context(tc.tile_pool(name="sd", bufs=2))
        tpool = ctx.enter_context(tc.tile_pool(name="tmp", bufs=2))
        pmpool = ctx.enter_context(tc.tile_pool(name="pmsb", bufs=2))
        ppool = ctx.enter_context(tc.tile_pool(name="pst", bufs=2))
        opool = ctx.enter_context(tc.tile_pool(name="osb", bufs=4))
        wpool = ctx.enter_context(tc.tile_pool(name="wsb", bufs=2))
        dpool = ctx.enter_context(tc.tile_pool(name="diag", bufs=1))
        pspool = ctx.enter_context(tc.tile_pool(name="ps", bufs=4, space="PSUM"))

        for ch in range(NCHUNK):
            g0 = ch * 128
            w_sb = wpool.tile([128, NW], F32, tag="wsb")
            nc.sync.dma_start(w_sb[:], w_d[g0 : g0 + 128, :])
            diag_f = dpool.tile([128, NW, 128], F32, tag="df")
            nc.gpsimd.affine_select(
                out=diag_f[:],
                in_=w_sb[:].broadcast_to([128, NW, 128]),
                compare_op=mybir.AluOpType.is_equal,
                fill=0.0,
                base=0,
                pattern=[[0, NW], [-1, 128]],
                channel_multiplier=1,
            )
            diag = dpool.tile([128, NW, 128], BF16, tag="db")
            nc.vector.tensor_copy(diag[:], diag_f[:])

            for ht in range(NHT):
                h0 = ht * HT
                is_f = _is_f_tile(ch, ht)
                xt = xpool.tile([128, HT + 2, WP], BF16, tag="xraw")
                nc.sync.dma_start(xt[:], x_d[g0 : g0 + 128, h0 : h0 + HT + 2, :])
                xc = xt[:, 1 : 1 + HT, 1 : 1 + W]

                def xa_xb(k):
                    (ai, aj), (bi, bj) = PAIRS[k]
                    return (xt[:, ai : ai + HT, aj : aj + W],
                            xt[:, bi : bi + HT, bj : bj + W])

                sd = spool.tile([128, 8, HT, W], BF16, tag="sd")
                if not is_f:
                    # PM tile: sd[0..3] = s_k, sd[4..7] = d_k
                    for k in range(4):
                        xa, xb = xa_xb(k)
                        nc.vector.tensor_tensor(out=sd[:, k], in0=xa, in1=xb, op=ADD)
                        nc.vector.tensor_tensor(out=sd[:, 4 + k], in0=xa, in1=xb, op=SUBT)
                else:
                    # F tile: sd[0..3] = pi1..pi4, sd[4..7] = d_k
                    tmp = tpool.tile([128, 4, HT, W], BF16, tag="tmp")
                    for k in range(4):
                        xa, xb = xa_xb(k)
                        nc.vector.tensor_tensor(out=tmp[:, k], in0=xa, in1=xb, op=ADD)
                        nc.vector.tensor_tensor(out=sd[:, 4 + k], in0=xa, in1=xb, op=SUBT)
                    nc.vector.tensor_tensor(out=sd[:, 0], in0=tmp[:, 0], in1=tmp[:, 2], op=ADD)
                    nc.vector.tensor_tensor(out=sd[:, 1], in0=tmp[:, 1], in1=tmp[:, 3], op=ADD)
                    nc.vector.tensor_tensor(out=sd[:, 2], in0=tmp[:, 0], in1=tmp[:, 2], op=SUBT)
                    nc.vector.tensor_tensor(out=sd[:, 3], in0=tmp[:, 1], in1=tmp[:, 3], op=SUBT)

                def chain(ps, half, cols, movs, s):
                    r0 = SUB * s
                    nmm = len(cols)
                    for i, (col, mv) in enumerate(zip(cols, movs)):
                        nc.tensor.matmul(
                            ps[:, half],
                            diag[:, col, :],
                            xc[:, r0 : r0 + SUB, :] if mv == "c"
                            else sd[:, mv, r0 : r0 + SUB, :],
                            start=(i == 0),
                            stop=(i == nmm - 1),
                        )

                if not is_f:
                    pairs_spec = [
                        (([0, 1, 2, 3, 4], [0, 1, 2, 3, "c"]), ([5, 6, 7, 8], [4, 5, 6, 7])),
                        (([9, 10, 11, 12, 13], [0, 1, 2, 3, "c"]), ([14, 15, 16, 17], [4, 5, 6, 7])),
                    ]
                else:
                    pairs_spec = [
                        (([18, 19, 20], [0, 1, "c"]), ([21, 22], [2, 3])),
                        (([5, 6, 7, 8], [4, 5, 6, 7]), ([14, 15, 16, 17], [4, 5, 6, 7])),
                    ]

                pms = []
                for pi, (specA, specB) in enumerate(pairs_spec):
                    pm = pmpool.tile([128, 2, HT, W], BF16, tag=f"pm{pi}")
                    pms.append(pm)
                    for s in range(NSUB):
                        ps = pspool.tile([128, 2, SUB, W], F32, tag="ps")
                        chain(ps, 0, specA[0], specA[1], s)
                        chain(ps, 1, specB[0], specB[1], s)
                        nc.scalar.activation(
                            pm[:, :, SUB * s : SUB * s + SUB, :],
                            ps[:],
                            mybir.ActivationFunctionType.Copy,
                        )

                def store(r, osb):
                    for p0, n, b_loc, c0 in CHUNK_SEGS[ch]:
                        row0 = b_loc * 4 * C + r * C + c0
                        nc.sync.dma_start(
                            o_d[row0 : row0 + n, h0 : h0 + HT, :],
                            osb[p0 : p0 + n, :, :],
                        )

                if not is_f:
                    # pms[0] = (p|m), pms[1] = (p'|m')
                    for pi, pm in enumerate(pms):
                        for sgn, r in ((0, pi), (1, pi + 2)):
                            osb = opool.tile([128, HT, W], BF16, tag="osb")
                            nc.vector.tensor_tensor(
                                out=osb[:], in0=pm[:, 0], in1=pm[:, 1],
                                op=ADD if sgn == 0 else SUBT,
                            )
                            store(r, osb)
                else:
                    # pms[0] = (qt|qs), pms[1] = (m|m')
                    pst = ppool.tile([128, 2, HT, W], BF16, tag="pst")
                    nc.vector.tensor_tensor(out=pst[:, 0], in0=pms[0][:, 0], in1=pms[0][:, 1], op=ADD)
                    nc.vector.tensor_tensor(out=pst[:, 1], in0=pms[0][:, 0], in1=pms[0][:, 1], op=SUBT)
                    for pi in range(2):
                        for sgn, r in ((0, pi), (1, pi + 2)):
                            osb = opool.tile([128, HT, W], BF16, tag="osb")
                            nc.vector.tensor_tensor(
                                out=osb[:], in0=pst[:, pi], in1=pms[1][:, pi],
                                op=ADD if sgn == 0 else SUBT,
                            )
                            store(r, osb)

    nc.compile()
    return nc


_NC = None


def _get_nc():
    global _NC
    if _NC is None:
        _NC = _build()
    return _NC


def _make_w23(weight):
    base = np.asarray(weight, dtype=np.float32)[:, 0]  # [C, 3, 3]
    K = [np.rot90(base, r, axes=(1, 2)) for r in range(4)]
    w = np.zeros((C, NW), dtype=np.float32)
    for k, ((ai, aj), _) in enumerate(PAIRS):
        a = 0.5 * (K[0][:, ai, aj] + K[2][:, ai, aj])
        b = 0.5 * (K[0][:, ai, aj] - K[2][:, ai, aj])
        ap_ = 0.5 * (K[1][:, ai, aj] + K[3][:, ai, aj])
        bp = 0.5 * (K[1][:, ai, aj] - K[3][:, ai, aj])
        w[:, k] = a
        w[:, 5 + k] = b
        w[:, 9 + k] = ap_
        w[:, 14 + k] = bp
        if k < 2:
            w[:, 18 + k] = 0.5 * (a + ap_)   # t1, t2
            w[:, 21 + k] = 0.5 * (a - ap_)   # g1, g2
    ctr = K[0][:, 1, 1]
    w[:, 4] = ctr
    w[:, 13] = ctr
    w[:, 20] = ctr
    return np.tile(w, (BS, 1))


def _make_in_maps(x, weight):
    x = np.asarray(x, dtype=np.float32)
    w23 = _make_w23(weight)
    xp = np.zeros((B * C, HP, WP), dtype=NPBF16)
    xp[:, 1 : 1 + H, 1 : 1 + W] = x.reshape(B * C, H, W).astype(NPBF16)
    xp = xp.reshape(NCORES, ROWS, HP, WP)
    return [{"xp": np.ascontiguousarray(xp[k]), "w23": w23} for k in range(NCORES)]


def kernel(x, weight):
    in_maps = _make_in_maps(x, weight)
    nc = _get_nc()
    res = run_bass_kernel_spmd(nc, in_maps, list(range(NCORES))).results
    out = np.empty((B, 4 * C, H, W), dtype=np.float32)
    for k in range(NCORES):
        out[BS * k : BS * (k + 1)] = (
            res[k]["out"].astype(np.float32).reshape(BS, 4 * C, H, W)
        )
    return out
